# revision 29
# baseline (speedup 1.0000x reference)
"""Trainium2 Bass kernel for nn_MoEModel (conv feature extractor + top-2 MoE).

Strategy (8 NeuronCores):
  - Data-parallel conv trunk: each core runs conv1/pool/conv2/pool on its
    16-image batch shard.  Both convs run as f32r matmuls (full fp32
    precision, 1 cycle/col) with block-diagonal stationary packing:
    conv1 packs 4 images per matmul (K=36, M=128), conv2 packs 2 images
    (K=64, M=128).  Image patches load as contiguous-row DMAs (3 fat
    descriptors per image instead of 186 thin ones).
  - AllGather of flattened features h [128, 12544] in bf16.
  - Expert-parallel MoE: core e holds expert e's weights [12544, 1000]
    pre-cast to bf16 on the host (halves the dominant HBM stream and
    removes 98 on-chip casts).  Gate logits are computed in full fp32
    (top-2 selection is numerically sensitive).  Each core masks its
    expert output by its top-2 gate weight; ReduceScatter(+) combines;
    each core softmaxes its 16-row shard.
"""

import numpy as np

from concourse import bass, bacc, mybir
from concourse.tile import TileContext
from concourse.masks import make_identity
from concourse.bass_utils import run_bass_kernel_spmd

F32 = mybir.dt.float32
F32R = mybir.dt.float32r
BF16 = mybir.dt.bfloat16
AX = mybir.AxisListType
ALU = mybir.AluOpType
ACTF = mybir.ActivationFunctionType

B = 128          # global batch
SH = 16          # batch shard per core
E = 8            # experts == cores
C = 1000         # classes
D = 14 * 14 * 64 # 12544 flattened features
NK = D // 128    # 98 contraction chunks
RG = [list(range(E))]

# W prefetch ring: tiles of WPACK chunks each ([128, WPACK*1000] bf16)
WPACK = 4
NWT = (NK + WPACK - 1) // WPACK   # 25 tiles (last holds 2 chunks)
WBUFS = 15                        # ring depth in packed tiles
# h chunk packing for the expert loop
HPACK = 7
NHT = NK // HPACK                 # 14 tiles


def _ap(tensor, offset, dims):
    return bass.AP(tensor=tensor, offset=offset, ap=dims)


def _conv_trunk(nc, tc, x16, w1sb, b1sb, w2sb, b2sb, idsb, h_locals,
                pump=None, mid=None):
    """conv1+pool+conv2+pool+transpose for the 16-image shard -> h_localb.

    All matmuls f32r (fp32 precision, 1 cyc/col).  conv1: block-diag
    stationary packs 4 images (K=9*4=36 rows, M=4*32=128 cols), psum at
    partition base 0.  conv2: block-diag packs 2 images (K=64, M=128).
    """
    with (
        tc.tile_pool(name="conv", bufs=1) as cv,
        tc.tile_pool(name="cps", bufs=2, space="PSUM") as cps,
    ):
        fm1s = {}

        def conv1_group(g):
            im = cv.tile([36, 3968], F32R, tag="im", bufs=2)
            for j in range(4):
                img = 4 * g + j
                for dy in range(3):
                    nc.scalar.dma_start(
                        im[9 * j + 3 * dy:9 * j + 3 * dy + 3, 0:3906],
                        _ap(x16, img * 4096 + 64 * dy, [[1, 3], [1, 3906]]),
                    )
            # conv1: 8 row-tiles over the 62x62 output grid
            c1a = cv.tile([128, 3844], F32, tag="c1a", bufs=1)
            imv = im[:].rearrange("p (y x) -> p y x", x=64)
            for t in range(8):
                y0 = 8 * t
                nr = min(8, 62 - y0)
                ps1 = cps.tile([128, 512], F32, tag="c1ps")
                nc.tensor.matmul(
                    ps1[0:128, 0:nr * 62],
                    w1sb[0:36, :],
                    imv[0:36, y0:y0 + nr, 0:62],
                    start=True, stop=True,
                )
                nc.scalar.activation(
                    c1a[:, y0 * 62:(y0 + nr) * 62], ps1[:, 0:nr * 62],
                    ACTF.Relu, bias=b1sb[:], scale=1.0,
                )
            # maxpool 2x2: 62x62 -> 31x31  (fm1 f32r for conv2)
            m1 = cv.tile([128, 62 * 31], F32, tag="m1", bufs=1)
            v = c1a[:].rearrange("p (y x) -> p y x", y=62)
            m1v = m1[:].rearrange("p (y x) -> p y x", y=62)
            nc.vector.tensor_max(m1v, v[:, :, 0:62:2], v[:, :, 1:62:2])
            fm1 = cv.tile([128, 961], F32R, tag="fm1", bufs=3)
            m1r = m1[:].rearrange("p (y x) -> p y x", y=62)
            fm1v = fm1[:].rearrange("p (y x) -> p y x", y=31)
            nc.vector.tensor_max(fm1v, m1r[:, 0:62:2, :], m1r[:, 1:62:2, :])
            fm1s[g] = fm1

        def conv2_group(g):
            # ---- conv2 (f32r), 2 images per matmul via block-diag ----
            fm1y = fm1s.pop(g)[:].rearrange("p (y x) -> p y x", y=31)
            for pr in range(2):     # image pairs (4g+2pr, 4g+2pr+1)
                c2a = cv.tile([128, 29 * 28], F32, tag="c2a", bufs=2)
                for (r0, nr) in ((0, 17), (17, 12)):
                    ps2 = cps.tile([128, 512], F32, tag=f"c2ps{pr}", bufs=2)
                    for tap in range(9):
                        dy, dx = tap // 3, tap % 3
                        rhs = fm1y[64 * pr:64 * pr + 64,
                                   r0 + dy:r0 + dy + nr,
                                   dx:dx + 28]
                        nc.tensor.matmul(
                            ps2[0:128, 0:nr * 28],
                            w2sb[64 * pr:64 * pr + 64,
                                 128 * tap:128 * tap + 128],
                            rhs,
                            start=(tap == 0), stop=(tap == 8),
                            tile_position=(64 * pr, 0),
                        )
                    nc.scalar.activation(
                        c2a[:, r0 * 28:(r0 + nr) * 28],
                        ps2[0:128, 0:nr * 28],
                        ACTF.Relu, bias=b2sb[:], scale=1.0,
                    )
                # maxpool 2x2 on 28x28 of the 29x29 grid -> 14x14 (2 imgs)
                m2 = cv.tile([128, 29 * 14], F32, tag="m2", bufs=1)
                cv2v = c2a[:].rearrange("p (y x) -> p y x", y=29)
                m2v = m2[:].rearrange("p (y x) -> p y x", y=29)
                nc.vector.tensor_max(m2v, cv2v[:, :, 0:28:2], cv2v[:, :, 1:28:2])
                fm2 = cv.tile([128, 196], F32, tag="fm2", bufs=2)
                m2r = m2[:].rearrange("p (y x) -> p y x", y=29)
                fm2v = fm2[:].rearrange("p (y x) -> p y x", y=14)
                nc.vector.tensor_max(fm2v, m2r[:, 0:28:2, :], m2r[:, 1:28:2, :])
                # transpose both images at once: [128(2x64ch), 98pix] -> [98, 128]
                hstp = cv.tile([98, 256], BF16, tag="hstp", bufs=2)
                for half in range(2):
                    pst = cps.tile([98, 128], F32, tag="pst")
                    nc.tensor.transpose(
                        pst[:],
                        fm2[:, 98 * half:98 * half + 98],
                        idsb[:, :],
                    )
                    nc.vector.tensor_copy(
                        hstp[:, 128 * half:128 * half + 128], pst[:]
                    )
                # hstp cols: (half, img-in-pair, ch) ; write h rows (pix-major)
                for i in range(2):
                    img = 4 * g + 2 * pr + i
                    hloc = h_locals[img // 8]
                    nc.sync.dma_start(
                        _ap(hloc[:].tensor,
                            hloc[:].offset + (img % 8) * D,
                            [[64, 98], [98 * 64, 2], [1, 64]]),
                        hstp[:].rearrange("p (h i c) -> p h i c", h=2, i=2)
                        [:, :, i, :],
                    )

        # interleave conv1(g+1) with conv2(g) so the PE never drains while
        # group g's ACT+pool chain resolves; AG of images 0-7 fires mid-conv.
        conv1_group(0)
        if pump is not None:
            pump()
        conv1_group(1)
        conv2_group(0)
        if pump is not None:
            pump()
        conv1_group(2)
        conv2_group(1)
        if mid is not None:
            mid()
        if pump is not None:
            pump()
        conv1_group(3)
        conv2_group(2)
        if pump is not None:
            pump()
        conv2_group(3)


def _phase45(nc, tc, do_w, do_rs, wbf_tiles, idsb, gwhi, gwlo, gbsb, besb,
             selsb, h_allA, h_allB, cc_in, cc_out, out16):
    # =========== gating + expert matmul (bf16) ===========
    # Software-pipelined: transposes + psum->SBUF copies (on the otherwise
    # idle scalar engine) run PIPE chunks ahead of the matmuls so the PE
    # never stalls.  Gate logits accumulate in fp32 PSUM from split-bf16
    # gate weights (gw = gwhi + gwlo exactly to ~2^-16), sharing the same
    # stationary h^T as the expert matmuls.
    PIPE = 3
    with (
        tc.tile_pool(name="hload", bufs=3) as hl,
        tc.tile_pool(name="hbfp", bufs=2 + PIPE) as hbfp,
        tc.tile_pool(name="gp", bufs=1) as gp,
        tc.tile_pool(name="eps", bufs=1, space="PSUM") as epp,
        tc.tile_pool(name="tps", bufs=3, space="PSUM") as tpp,
    ):
        pse_a = epp.tile([128, 512], F32, tag="pse_a")
        pse_b = epp.tile([128, 488], F32, tag="pse_b")
        if not do_w:
            nc.tensor.matmul(pse_a[:, 0:128], idsb[:], idsb[:],
                             start=True, stop=True)
            nc.tensor.matmul(pse_b[:, 0:128], idsb[:], idsb[:],
                             start=True, stop=True)
        idb = gp.tile([128, 128], BF16, tag="idb")
        nc.vector.tensor_copy(idb[:], idsb[:])
        psg = epp.tile([128, 8], F32, tag="psg")

        hbts = {}
        hbfs = {}

        def _stage(k):
            """DMA (packed), transpose and copy chunk k."""
            if k >= NK:
                return
            ht, kk = divmod(k, HPACK)
            if kk == 0:
                hbt = hl.tile([128, HPACK * 128], BF16, tag="hb")
                c0, c1 = ht * HPACK * 128, (ht + 1) * HPACK * 128
                nc.scalar.dma_start(hbt[0:64, :], h_allA[:, c0:c1])
                nc.scalar.dma_start(hbt[64:128, :], h_allB[:, c0:c1])
                hbts[ht] = hbt
            hb = hbts[ht][:, kk * 128:(kk + 1) * 128]
            pt = tpp.tile([128, 128], BF16, tag="pt")
            nc.tensor.transpose(pt[:], hb, idb[:])
            hbf = hbfp.tile([128, 128], BF16, tag="hbf")
            nc.scalar.activation(hbf[:], pt[:], ACTF.Copy)
            hbfs[k] = hbf

        for k in range(PIPE):
            _stage(k)
        for k in range(NK):
            hbf = hbfs.pop(k)
            nc.tensor.matmul(
                psg[:], hbf[:], gwhi[:, k * 8:(k + 1) * 8],
                start=(k == 0), stop=False,
            )
            nc.tensor.matmul(
                psg[:], hbf[:], gwlo[:, k * 8:(k + 1) * 8],
                start=False, stop=(k == NK - 1),
            )
            if do_w:
                wt, kw = divmod(k, WPACK)
                wb = wbf_tiles[wt]
                nc.tensor.matmul(
                    pse_a[:], hbf[:], wb[:, kw * 1000:kw * 1000 + 512],
                    start=(k == 0), stop=(k == NK - 1),
                )
                nc.tensor.matmul(
                    pse_b[:], hbf[:], wb[:, kw * 1000 + 512:(kw + 1) * 1000],
                    start=(k == 0), stop=(k == NK - 1),
                )
            _stage(k + PIPE)

        # ---- gate softmax + top-2 mask (all [128, 8] fp32) ----
        g0 = gp.tile([128, 8], F32, tag="g0")
        nc.vector.tensor_add(g0[:], psg[:], gbsb[:])
        gmax = gp.tile([128, 1], F32, tag="gmax")
        nc.vector.reduce_max(gmax[:], g0[:], axis=AX.X)
        gmn = gp.tile([128, 1], F32, tag="gmn")
        nc.vector.tensor_scalar_mul(gmn[:], gmax[:], -1.0)
        gexp = gp.tile([128, 8], F32, tag="gexp")
        gsum = gp.tile([128, 1], F32, tag="gsum")
        nc.scalar.activation(
            gexp[:], g0[:], ACTF.Exp,
            bias=gmn[:], scale=1.0, accum_out=gsum[:],
        )
        grec = gp.tile([128, 1], F32, tag="grec")
        nc.vector.reciprocal(grec[:], gsum[:])
        gg = gp.tile([128, 8], F32, tag="gg")
        nc.vector.tensor_scalar_mul(gg[:], gexp[:], grec[:])
        m1t = gp.tile([128, 1], F32, tag="m1t")
        nc.vector.reduce_max(m1t[:], gg[:], axis=AX.X)
        negsel = gp.tile([128, 8], F32, tag="negsel")
        nc.vector.tensor_scalar(
            negsel[:], gg[:], m1t[:], -2.0,
            op0=ALU.is_equal, op1=ALU.mult,
        )
        masked = gp.tile([128, 8], F32, tag="masked")
        nc.vector.tensor_add(masked[:], gg[:], negsel[:])
        m2t = gp.tile([128, 1], F32, tag="m2t")
        nc.vector.reduce_max(m2t[:], masked[:], axis=AX.X)
        gsel = gp.tile([128, 8], F32, tag="gsel")
        nc.vector.tensor_mul(gsel[:], gg[:], selsb[:])
        ge = gp.tile([128, 1], F32, tag="ge")
        nc.vector.reduce_sum(ge[:], gsel[:], axis=AX.X)
        selm = gp.tile([128, 1], F32, tag="selm")
        nc.vector.tensor_scalar(
            selm[:], ge[:], m2t[:], None, op0=ALU.is_ge,
        )
        wsel = gp.tile([128, 1], F32, tag="wsel")
        nc.vector.tensor_mul(wsel[:], ge[:], selm[:])

        # ---- weighted contribution -> ReduceScatter ----
        contrib = gp.tile([128, C], F32, tag="contrib")
        nc.vector.tensor_add(contrib[:, 0:512], pse_a[:], besb[:, 0:512])
        nc.vector.tensor_add(contrib[:, 512:C], pse_b[:], besb[:, 512:C])
        nc.vector.tensor_scalar_mul(contrib[:], contrib[:], wsel[:])
        # permuted store: row p = 64h + 8c + i  ->  global sample 16c + 8h + i
        # (3-dim dst AP with an unsplit partition src lowers correctly)
        for h in range(2):
            nc.scalar.dma_start(
                _ap(cc_in[:].tensor, cc_in[:].offset + 8 * h * C,
                    [[16 * C, 8], [C, 8], [1, C]]),
                contrib[64 * h:64 * h + 64, :],
            )
        if do_rs:
            nc.gpsimd.collective_compute(
                "ReduceScatter", ALU.add, replica_groups=RG,
                ins=[cc_in.opt()], outs=[cc_out.opt()],
            )

        # ---- final softmax on the 16-row shard ----
        fin = gp.tile([SH, C], F32, tag="fin")
        nc.scalar.dma_start(fin[:], cc_out[:] if do_rs else cc_in[0:SH, :])
        fmax = gp.tile([SH, 1], F32, tag="fmax")
        nc.vector.reduce_max(fmax[:], fin[:], axis=AX.X)
        fmn = gp.tile([SH, 1], F32, tag="fmn")
        nc.vector.tensor_scalar_mul(fmn[:], fmax[:], -1.0)
        fexp = gp.tile([SH, C], F32, tag="fexp")
        fsum = gp.tile([SH, 1], F32, tag="fsum")
        nc.scalar.activation(
            fexp[:], fin[:], ACTF.Exp,
            bias=fmn[:], scale=1.0, accum_out=fsum[:],
        )
        frec = gp.tile([SH, 1], F32, tag="frec")
        nc.vector.reciprocal(frec[:], fsum[:])
        fout = gp.tile([SH, C], F32, tag="fout")
        nc.vector.tensor_scalar_mul(fout[:], fexp[:], frec[:])
        nc.scalar.dma_start(out16[:], fout[:])


def build_program(variant="full", repeat=1):
    do_conv = variant not in ("no_conv", "no_conv_no_ag", "expert_only")
    do_ag = variant not in ("no_ag", "no_conv_no_ag", "expert_only", "conv_only")
    do_w = variant not in ("no_expert", "conv_only")
    do_rs = variant not in ("no_rs", "expert_only")
    nc = bacc.Bacc("TRN2", target_bir_lowering=False, debug=False, num_devices=E)

    # ---- per-core external I/O ----
    x16 = nc.dram_tensor("x16", [SH, 4096], F32R, kind="ExternalInput")
    w1bd = nc.dram_tensor("w1bd", [36, 128], F32R, kind="ExternalInput")
    b1 = nc.dram_tensor("b1", [128, 1], F32, kind="ExternalInput")
    w2bd = nc.dram_tensor("w2bd", [128, 1152], F32R, kind="ExternalInput")
    b2 = nc.dram_tensor("b2", [128, 1], F32, kind="ExternalInput")
    gwhid = nc.dram_tensor("gwhid", [128, NK * 8], BF16, kind="ExternalInput")
    gwlod = nc.dram_tensor("gwlod", [128, NK * 8], BF16, kind="ExternalInput")
    gb128 = nc.dram_tensor("gb128", [128, 8], F32, kind="ExternalInput")
    web = nc.dram_tensor("web", [NWT, 128, WPACK * 1000], BF16,
                         kind="ExternalInput")
    be128 = nc.dram_tensor("be128", [128, C], F32, kind="ExternalInput")
    sel = nc.dram_tensor("sel", [128, 8], F32, kind="ExternalInput")
    out16 = nc.dram_tensor("out16", [SH, C], F32, kind="ExternalOutput")

    with TileContext(nc) as tc:
        with (
            tc.tile_pool(name="consts", bufs=1) as cp,
            tc.tile_pool(name="wbf", bufs=WBUFS) as wbf,
            tc.tile_pool(name="dram", bufs=1, space="DRAM") as dp,
        ):
            # ---- constants into SBUF (scalar: conv-critical; sync: rest) ----
            w1sb = cp.tile([36, 128], F32R, tag="w1sb")
            nc.scalar.dma_start(w1sb[:], w1bd[:, :])
            b1sb = cp.tile([128, 1], F32, tag="b1sb")
            nc.scalar.dma_start(b1sb[:], b1[:, :])
            b2sb = cp.tile([128, 1], F32, tag="b2sb")
            nc.scalar.dma_start(b2sb[:], b2[:, :])
            w2sb = cp.tile([128, 1152], F32R, tag="w2sb")
            nc.sync.dma_start(w2sb[:], w2bd[:, :])
            idsb = cp.tile([128, 128], F32, tag="idsb")
            make_identity(nc, idsb[:])
            # gate weights host-rearranged: col k*8+j = gw[128k+p, j],
            # split hi/lo bf16 (gwhi + gwlo == gw to ~2^-16)
            gwhi = cp.tile([128, NK * 8], BF16, tag="gwhi")
            nc.sync.dma_start(gwhi[:], gwhid[:, :])
            gwlo = cp.tile([128, NK * 8], BF16, tag="gwlo")
            nc.sync.dma_start(gwlo[:], gwlod[:, :])
            gbsb = cp.tile([128, 8], F32, tag="gbsb")
            nc.sync.dma_start(gbsb[:], gb128[:, :])
            besb = cp.tile([128, C], F32, tag="besb")
            nc.sync.dma_start(besb[:], be128[:, :])
            selsb = cp.tile([128, 8], F32, tag="selsb")
            nc.sync.dma_start(selsb[:], sel[:, :])

            # ---- DRAM bounce buffers for collectives ----
            h_localA = dp.tile([SH // 2, D], BF16, tag="h_localA")
            h_localB = dp.tile([SH // 2, D], BF16, tag="h_localB")
            cc_in = dp.tile([B, C], F32, tag="cc_in")
            cc_out = dp.tile([SH, C], F32, tag="cc_out")

            for _rep in range(repeat):
                h_allA = dp.tile([B // 2, D], BF16, tag=f"h_allA{_rep}",
                                 addr_space="Shared")
                h_allB = dp.tile([B // 2, D], BF16, tag=f"h_allB{_rep}",
                                 addr_space="Shared")
                # ---- expert weight stream: packed bf16 tiles ----
                wbf_tiles = []
                _pumped = [0]

                def pump(n=3, _rep=_rep):
                    if not do_w:
                        return
                    hi = min(NWT, _pumped[0] + n)
                    for t in range(_pumped[0], hi):
                        wb = wbf.tile([128, WPACK * 1000], BF16, tag="wb")
                        nc.sync.dma_start(wb[:], web[t, :, :])
                        wbf_tiles.append(wb)
                    _pumped[0] = hi

                def ag_first(_rep=_rep):
                    # images 0-7 of every core are done: gather them while
                    # the conv trunk works on images 8-15.
                    if do_ag:
                        nc.gpsimd.collective_compute(
                            "AllGather", ALU.bypass, replica_groups=RG,
                            ins=[h_localA.opt()], outs=[h_allA.opt()],
                        )

                # =========== conv trunk ===========
                if do_conv:
                    _conv_trunk(nc, tc, x16, w1sb, b1sb, w2sb, b2sb, idsb,
                                (h_localA, h_localB),
                                pump=lambda: pump(3), mid=ag_first)
                else:
                    ag_first()
                pump(NWT)

                # =========== AllGather h (bf16), second half ===========
                if do_ag:
                    nc.gpsimd.collective_compute(
                        "AllGather", ALU.bypass, replica_groups=RG,
                        ins=[h_localB.opt()], outs=[h_allB.opt()],
                    )

                if variant not in ("conv_only", "conv_ag"):
                    _phase45(nc, tc, do_w, do_rs, wbf_tiles, idsb, gwhi,
                             gwlo, gbsb, besb, selsb, h_allA, h_allB,
                             cc_in, cc_out, out16)

    nc.compile()
    return nc


_NC_CACHE = None


def _get_program():
    global _NC_CACHE
    if _NC_CACHE is None:
        _NC_CACHE = build_program()
    return _NC_CACHE


def _pack_web(expert_w_e):
    """[D, C] fp32 -> [NWT, 128, WPACK*1000] bf16 (chunks packed along rows)."""
    import ml_dtypes
    ew = np.asarray(expert_w_e, np.float32).astype(ml_dtypes.bfloat16)
    out = np.zeros((NWT, 128, WPACK * 1000), ml_dtypes.bfloat16)
    for t in range(NWT):
        for i in range(WPACK):
            k = t * WPACK + i
            if k >= NK:
                break
            out[t, :, i * 1000:(i + 1) * 1000] = ew[k * 128:(k + 1) * 128, :]
    return out


def make_in_maps(x, conv1_w, conv1_b, conv2_w, conv2_b,
                 gate_w, gate_b, expert_w, expert_b):
    x = np.asarray(x, np.float32).reshape(B, 4096)
    w1 = np.asarray(conv1_w, np.float32).reshape(9, 32)
    w1bd = np.zeros((36, 128), np.float32)
    for j in range(4):
        w1bd[9 * j:9 * j + 9, 32 * j:32 * j + 32] = w1
    b1 = np.ascontiguousarray(
        np.tile(np.asarray(conv1_b, np.float32), 4).reshape(128, 1))
    # conv2 block-diag-2: per tap [64, 128]: rows 0:32 img-even in-ch
    # (cols 0:64 out), rows 32:64 img-odd; duplicated to rows 64:128.
    w2 = np.asarray(conv2_w, np.float32).reshape(9, 32, 64)
    w2bd = np.zeros((128, 1152), np.float32)
    for tap in range(9):
        blk = np.zeros((64, 128), np.float32)
        blk[0:32, 0:64] = w2[tap]
        blk[32:64, 64:128] = w2[tap]
        w2bd[0:64, 128 * tap:128 * tap + 128] = blk
        w2bd[64:128, 128 * tap:128 * tap + 128] = blk
    b2 = np.ascontiguousarray(
        np.tile(np.asarray(conv2_b, np.float32), 2).reshape(128, 1))
    # gate weights pre-arranged [128, 98*8]: col k*8+j = gw[128k+p, j],
    # split into hi/lo bf16 halves (exact to ~2^-16 relative)
    import ml_dtypes
    gwf = np.asarray(gate_w, np.float32)
    gwre = np.ascontiguousarray(
        gwf.reshape(NK, 128, 8).transpose(1, 0, 2).reshape(128, NK * 8))
    gwhi = gwre.astype(ml_dtypes.bfloat16)
    gwlo = (gwre - gwhi.astype(np.float32)).astype(ml_dtypes.bfloat16)
    gb128 = np.ascontiguousarray(
        np.broadcast_to(np.asarray(gate_b, np.float32), (128, 8)))
    ew = np.asarray(expert_w, np.float32)
    eb = np.asarray(expert_b, np.float32)
    in_maps = []
    for r in range(E):
        onehot = np.zeros((1, 8), np.float32)
        onehot[0, r] = 1.0
        in_maps.append({
            "x16": np.ascontiguousarray(x[r * SH:(r + 1) * SH]),
            "w1bd": w1bd, "b1": b1, "w2bd": w2bd, "b2": b2,
            "gwhid": gwhi, "gwlod": gwlo, "gb128": gb128,
            "web": _pack_web(ew[r]),
            "be128": np.ascontiguousarray(
                np.broadcast_to(eb[r], (128, C))),
            "sel": np.ascontiguousarray(np.broadcast_to(onehot, (128, 8))),
        })
    return in_maps


def kernel(**inputs):
    nc = _get_program()
    in_maps = make_in_maps(**inputs)
    res = run_bass_kernel_spmd(nc, in_maps, core_ids=list(range(E)))
    return np.concatenate([res.results[r]["out16"] for r in range(E)], axis=0)


# revision 36
# speedup vs baseline: 413.9780x; 413.9780x over previous
"""Trainium2 Bass kernel for nn_MoEModel (conv feature extractor + top-2 MoE).

Strategy (8 NeuronCores):
  - Data-parallel conv trunk: each core runs conv1/pool/conv2/pool on its
    16-image batch shard.  Both convs run as f32r matmuls (full fp32
    precision, 1 cycle/col) with block-diagonal stationary packing:
    conv1 packs 4 images per matmul (K=36, M=128), conv2 packs 2 images
    (K=64, M=128).  Image patches load as contiguous-row DMAs (3 fat
    descriptors per image instead of 186 thin ones).
  - AllGather of flattened features h [128, 12544] in bf16.
  - Expert-parallel MoE: core e holds expert e's weights [12544, 1000]
    pre-cast to bf16 on the host (halves the dominant HBM stream and
    removes 98 on-chip casts).  Gate logits are computed in full fp32
    (top-2 selection is numerically sensitive).  Each core masks its
    expert output by its top-2 gate weight; ReduceScatter(+) combines;
    each core softmaxes its 16-row shard.
"""

import numpy as np

from concourse import bass, bacc, mybir
from concourse.tile import TileContext
from concourse.masks import make_identity
from concourse.bass_utils import run_bass_kernel_spmd

F32 = mybir.dt.float32
F32R = mybir.dt.float32r
BF16 = mybir.dt.bfloat16
AX = mybir.AxisListType
ALU = mybir.AluOpType
ACTF = mybir.ActivationFunctionType

B = 128          # global batch
SH = 16          # batch shard per core
E = 8            # experts == cores
C = 1000         # classes
D = 14 * 14 * 64 # 12544 flattened features
NK = D // 128    # 98 contraction chunks
RG = [list(range(E))]

# W prefetch ring: tiles of WPACK chunks each ([128, WPACK*1000] bf16)
WPACK = 4
NWT = (NK + WPACK - 1) // WPACK   # 25 tiles (last holds 2 chunks)
WBUFS = 15                        # ring depth in packed tiles
# h chunk packing for the expert loop
HPACK = 7
NHT = NK // HPACK                 # 14 tiles


def _ap(tensor, offset, dims):
    return bass.AP(tensor=tensor, offset=offset, ap=dims)


def _conv_trunk(nc, tc, x16, w1sb, b1sb, w2sb, b2sb, idsb, h_locals,
                pump=None, mid=None):
    """conv1+pool+conv2+pool+transpose for the 16-image shard -> h_localb.

    All matmuls f32r (fp32 precision, 1 cyc/col).  conv1: block-diag
    stationary packs 4 images (K=9*4=36 rows, M=4*32=128 cols), psum at
    partition base 0.  conv2: block-diag packs 2 images (K=64, M=128).
    """
    with (
        tc.tile_pool(name="conv", bufs=1) as cv,
        tc.tile_pool(name="cps", bufs=2, space="PSUM") as cps,
    ):
        fm1s = {}

        def conv1_group(g):
            im = cv.tile([36, 3968], F32R, tag="im", bufs=2)
            for j in range(4):
                img = 4 * g + j
                for dy in range(3):
                    nc.scalar.dma_start(
                        im[9 * j + 3 * dy:9 * j + 3 * dy + 3, 0:3906],
                        _ap(x16, img * 4096 + 64 * dy, [[1, 3], [1, 3906]]),
                    )
            # conv1: 8 row-tiles over the 62x62 output grid
            c1a = cv.tile([128, 3844], F32, tag="c1a", bufs=1)
            imv = im[:].rearrange("p (y x) -> p y x", x=64)
            for t in range(8):
                y0 = 8 * t
                nr = min(8, 62 - y0)
                ps1 = cps.tile([128, 512], F32, tag="c1ps")
                nc.tensor.matmul(
                    ps1[0:128, 0:nr * 62],
                    w1sb[0:36, :],
                    imv[0:36, y0:y0 + nr, 0:62],
                    start=True, stop=True,
                )
                nc.scalar.activation(
                    c1a[:, y0 * 62:(y0 + nr) * 62], ps1[:, 0:nr * 62],
                    ACTF.Relu, bias=b1sb[:], scale=1.0,
                )
            # maxpool 2x2: 62x62 -> 31x31  (fm1 f32r for conv2)
            m1 = cv.tile([128, 62 * 31], F32, tag="m1", bufs=1)
            v = c1a[:].rearrange("p (y x) -> p y x", y=62)
            m1v = m1[:].rearrange("p (y x) -> p y x", y=62)
            nc.vector.tensor_max(m1v, v[:, :, 0:62:2], v[:, :, 1:62:2])
            fm1 = cv.tile([128, 961], F32R, tag="fm1", bufs=3)
            m1r = m1[:].rearrange("p (y x) -> p y x", y=62)
            fm1v = fm1[:].rearrange("p (y x) -> p y x", y=31)
            nc.vector.tensor_max(fm1v, m1r[:, 0:62:2, :], m1r[:, 1:62:2, :])
            fm1s[g] = fm1

        def conv2_group(g):
            # ---- conv2 (f32r), 2 images per matmul via block-diag ----
            fm1y = fm1s.pop(g)[:].rearrange("p (y x) -> p y x", y=31)
            for pr in range(2):     # image pairs (4g+2pr, 4g+2pr+1)
                c2a = cv.tile([128, 29 * 28], F32, tag="c2a", bufs=2)
                for (r0, nr) in ((0, 17), (17, 12)):
                    ps2 = cps.tile([128, 512], F32, tag=f"c2ps{pr}", bufs=2)
                    for tap in range(9):
                        dy, dx = tap // 3, tap % 3
                        rhs = fm1y[64 * pr:64 * pr + 64,
                                   r0 + dy:r0 + dy + nr,
                                   dx:dx + 28]
                        nc.tensor.matmul(
                            ps2[0:128, 0:nr * 28],
                            w2sb[64 * pr:64 * pr + 64,
                                 128 * tap:128 * tap + 128],
                            rhs,
                            start=(tap == 0), stop=(tap == 8),
                            tile_position=(64 * pr, 0),
                        )
                    nc.scalar.activation(
                        c2a[:, r0 * 28:(r0 + nr) * 28],
                        ps2[0:128, 0:nr * 28],
                        ACTF.Relu, bias=b2sb[:], scale=1.0,
                    )
                # maxpool 2x2 on 28x28 of the 29x29 grid -> 14x14 (2 imgs)
                m2 = cv.tile([128, 29 * 14], F32, tag="m2", bufs=1)
                cv2v = c2a[:].rearrange("p (y x) -> p y x", y=29)
                m2v = m2[:].rearrange("p (y x) -> p y x", y=29)
                nc.vector.tensor_max(m2v, cv2v[:, :, 0:28:2], cv2v[:, :, 1:28:2])
                fm2 = cv.tile([128, 196], F32, tag="fm2", bufs=2)
                m2r = m2[:].rearrange("p (y x) -> p y x", y=29)
                fm2v = fm2[:].rearrange("p (y x) -> p y x", y=14)
                nc.vector.tensor_max(fm2v, m2r[:, 0:28:2, :], m2r[:, 1:28:2, :])
                # transpose both images at once: [128(2x64ch), 98pix] -> [98, 128]
                hstp = cv.tile([98, 256], BF16, tag="hstp", bufs=2)
                for half in range(2):
                    pst = cps.tile([98, 128], F32, tag="pst")
                    nc.tensor.transpose(
                        pst[:],
                        fm2[:, 98 * half:98 * half + 98],
                        idsb[:, :],
                    )
                    nc.vector.tensor_copy(
                        hstp[:, 128 * half:128 * half + 128], pst[:]
                    )
                # hstp cols: (half, img-in-pair, ch) ; write h rows (pix-major)
                for i in range(2):
                    img = 4 * g + 2 * pr + i
                    blk = 0 if img < 8 else (1 if img < 12 else 2)
                    hloc = h_locals[blk]
                    rel = img - (0, 8, 12)[blk]
                    nc.sync.dma_start(
                        _ap(hloc[:].tensor,
                            hloc[:].offset + rel * D,
                            [[64, 98], [98 * 64, 2], [1, 64]]),
                        hstp[:].rearrange("p (h i c) -> p h i c", h=2, i=2)
                        [:, :, i, :],
                    )

        # interleave conv1(g+1) with conv2(g) so the PE never drains while
        # group g's ACT+pool chain resolves; partial AllGathers fire as
        # image blocks complete (0-7, 8-11; the final 12-15 AG is emitted
        # by the caller).
        conv1_group(0)
        if pump is not None:
            pump()
        conv1_group(1)
        conv2_group(0)
        if pump is not None:
            pump()
        conv1_group(2)
        conv2_group(1)
        if mid is not None:
            mid(0)
        if pump is not None:
            pump()
        conv1_group(3)
        conv2_group(2)
        if mid is not None:
            mid(1)
        if pump is not None:
            pump()
        conv2_group(3)


def _phase45(nc, tc, do_w, do_rs, wbf_tiles, idsb, gwhi, gwlo, gbsb, besb,
             selsb, h_alls, cc_in, cc_out, out16):
    # =========== gating + expert matmul (bf16) ===========
    # Software-pipelined: transposes + psum->SBUF copies (on the otherwise
    # idle scalar engine) run PIPE chunks ahead of the matmuls so the PE
    # never stalls.  Gate logits accumulate in fp32 PSUM from split-bf16
    # gate weights (gw = gwhi + gwlo exactly to ~2^-16), sharing the same
    # stationary h^T as the expert matmuls.
    PIPE = 3
    with (
        tc.tile_pool(name="hload", bufs=3) as hl,
        tc.tile_pool(name="hbfp", bufs=2 + PIPE) as hbfp,
        tc.tile_pool(name="gp", bufs=1) as gp,
        tc.tile_pool(name="eps", bufs=1, space="PSUM") as epp,
        tc.tile_pool(name="tps", bufs=3, space="PSUM") as tpp,
    ):
        pse_a = epp.tile([128, 512], F32, tag="pse_a")
        pse_b = epp.tile([128, 488], F32, tag="pse_b")
        if not do_w:
            nc.tensor.matmul(pse_a[:, 0:128], idsb[:], idsb[:],
                             start=True, stop=True)
            nc.tensor.matmul(pse_b[:, 0:128], idsb[:], idsb[:],
                             start=True, stop=True)
        idb = gp.tile([128, 128], BF16, tag="idb")
        nc.vector.tensor_copy(idb[:], idsb[:])
        psg = epp.tile([128, 8], F32, tag="psg")

        hbts = {}
        hbfs = {}

        def _stage(k):
            """DMA (packed), transpose and copy chunk k."""
            if k >= NK:
                return
            ht, kk = divmod(k, HPACK)
            if kk == 0:
                hbt = hl.tile([128, HPACK * 128], BF16, tag="hb")
                c0, c1 = ht * HPACK * 128, (ht + 1) * HPACK * 128
                nc.scalar.dma_start(hbt[0:64, :], h_alls[0][:, c0:c1])
                nc.scalar.dma_start(hbt[64:96, :], h_alls[1][:, c0:c1])
                nc.scalar.dma_start(hbt[96:128, :], h_alls[2][:, c0:c1])
                hbts[ht] = hbt
            hb = hbts[ht][:, kk * 128:(kk + 1) * 128]
            pt = tpp.tile([128, 128], BF16, tag="pt")
            nc.tensor.transpose(pt[:], hb, idb[:])
            hbf = hbfp.tile([128, 128], BF16, tag="hbf")
            nc.scalar.activation(hbf[:], pt[:], ACTF.Copy)
            hbfs[k] = hbf

        for k in range(PIPE):
            _stage(k)
        for k in range(NK):
            hbf = hbfs.pop(k)
            nc.tensor.matmul(
                psg[:], hbf[:], gwhi[:, k * 8:(k + 1) * 8],
                start=(k == 0), stop=False,
            )
            nc.tensor.matmul(
                psg[:], hbf[:], gwlo[:, k * 8:(k + 1) * 8],
                start=False, stop=(k == NK - 1),
            )
            if do_w:
                wt, kw = divmod(k, WPACK)
                wb = wbf_tiles[wt]
                nc.tensor.matmul(
                    pse_a[:], hbf[:], wb[:, kw * 1000:kw * 1000 + 512],
                    start=(k == 0), stop=(k == NK - 1),
                )
                nc.tensor.matmul(
                    pse_b[:], hbf[:], wb[:, kw * 1000 + 512:(kw + 1) * 1000],
                    start=(k == 0), stop=(k == NK - 1),
                )
            _stage(k + PIPE)

        # ---- gate softmax + top-2 mask (all [128, 8] fp32) ----
        g0 = gp.tile([128, 8], F32, tag="g0")
        nc.vector.tensor_add(g0[:], psg[:], gbsb[:])
        gmax = gp.tile([128, 1], F32, tag="gmax")
        nc.vector.reduce_max(gmax[:], g0[:], axis=AX.X)
        gmn = gp.tile([128, 1], F32, tag="gmn")
        nc.vector.tensor_scalar_mul(gmn[:], gmax[:], -1.0)
        gexp = gp.tile([128, 8], F32, tag="gexp")
        gsum = gp.tile([128, 1], F32, tag="gsum")
        nc.scalar.activation(
            gexp[:], g0[:], ACTF.Exp,
            bias=gmn[:], scale=1.0, accum_out=gsum[:],
        )
        grec = gp.tile([128, 1], F32, tag="grec")
        nc.vector.reciprocal(grec[:], gsum[:])
        gg = gp.tile([128, 8], F32, tag="gg")
        nc.vector.tensor_scalar_mul(gg[:], gexp[:], grec[:])
        m1t = gp.tile([128, 1], F32, tag="m1t")
        nc.vector.reduce_max(m1t[:], gg[:], axis=AX.X)
        negsel = gp.tile([128, 8], F32, tag="negsel")
        nc.vector.tensor_scalar(
            negsel[:], gg[:], m1t[:], -2.0,
            op0=ALU.is_equal, op1=ALU.mult,
        )
        masked = gp.tile([128, 8], F32, tag="masked")
        nc.vector.tensor_add(masked[:], gg[:], negsel[:])
        m2t = gp.tile([128, 1], F32, tag="m2t")
        nc.vector.reduce_max(m2t[:], masked[:], axis=AX.X)
        gsel = gp.tile([128, 8], F32, tag="gsel")
        nc.vector.tensor_mul(gsel[:], gg[:], selsb[:])
        ge = gp.tile([128, 1], F32, tag="ge")
        nc.vector.reduce_sum(ge[:], gsel[:], axis=AX.X)
        selm = gp.tile([128, 1], F32, tag="selm")
        nc.vector.tensor_scalar(
            selm[:], ge[:], m2t[:], None, op0=ALU.is_ge,
        )
        wsel = gp.tile([128, 1], F32, tag="wsel")
        nc.vector.tensor_mul(wsel[:], ge[:], selm[:])

        # ---- weighted contribution -> ReduceScatter ----
        contrib = gp.tile([128, C], F32, tag="contrib")
        nc.vector.tensor_add(contrib[:, 0:512], pse_a[:], besb[:, 0:512])
        nc.vector.tensor_add(contrib[:, 512:C], pse_b[:], besb[:, 512:C])
        nc.vector.tensor_scalar_mul(contrib[:], contrib[:], wsel[:])
        # permuted store (3-dim dst AP, unsplit partition src):
        #   p in [0,64):   core c=p//8, img p%8    -> row 16c + p%8
        #   p in [64,96):  core c=(p-64)//4, img 8+(p-64)%4  -> row 16c+8+...
        #   p in [96,128): core c=(p-96)//4, img 12+(p-96)%4 -> row 16c+12+...
        for (p0, ni, base) in ((0, 8, 0), (64, 4, 8), (96, 4, 12)):
            nc.scalar.dma_start(
                _ap(cc_in[:].tensor, cc_in[:].offset + base * C,
                    [[16 * C, 8], [C, ni], [1, C]]),
                contrib[p0:p0 + 8 * ni, :],
            )
        if do_rs:
            nc.gpsimd.collective_compute(
                "ReduceScatter", ALU.add, replica_groups=RG,
                ins=[cc_in.opt()], outs=[cc_out.opt()],
            )

        # ---- final softmax on the 16-row shard ----
        fin = gp.tile([SH, C], F32, tag="fin")
        nc.scalar.dma_start(fin[:], cc_out[:] if do_rs else cc_in[0:SH, :])
        fmax = gp.tile([SH, 1], F32, tag="fmax")
        nc.vector.reduce_max(fmax[:], fin[:], axis=AX.X)
        fmn = gp.tile([SH, 1], F32, tag="fmn")
        nc.vector.tensor_scalar_mul(fmn[:], fmax[:], -1.0)
        fexp = gp.tile([SH, C], F32, tag="fexp")
        fsum = gp.tile([SH, 1], F32, tag="fsum")
        nc.scalar.activation(
            fexp[:], fin[:], ACTF.Exp,
            bias=fmn[:], scale=1.0, accum_out=fsum[:],
        )
        frec = gp.tile([SH, 1], F32, tag="frec")
        nc.vector.reciprocal(frec[:], fsum[:])
        fout = gp.tile([SH, C], F32, tag="fout")
        nc.vector.tensor_scalar_mul(fout[:], fexp[:], frec[:])
        nc.scalar.dma_start(out16[:], fout[:])


def build_program(variant="full", repeat=1):
    do_conv = variant not in ("no_conv", "no_conv_no_ag", "expert_only")
    do_ag = variant not in ("no_ag", "no_conv_no_ag", "expert_only", "conv_only")
    do_w = variant not in ("no_expert", "conv_only")
    do_rs = variant not in ("no_rs", "expert_only")
    nc = bacc.Bacc("TRN2", target_bir_lowering=False, debug=False, num_devices=E)

    # ---- per-core external I/O ----
    x16 = nc.dram_tensor("x16", [SH, 4096], F32R, kind="ExternalInput")
    w1bd = nc.dram_tensor("w1bd", [36, 128], F32R, kind="ExternalInput")
    b1 = nc.dram_tensor("b1", [128, 1], F32, kind="ExternalInput")
    w2bd = nc.dram_tensor("w2bd", [128, 1152], F32R, kind="ExternalInput")
    b2 = nc.dram_tensor("b2", [128, 1], F32, kind="ExternalInput")
    gwhid = nc.dram_tensor("gwhid", [128, NK * 8], BF16, kind="ExternalInput")
    gwlod = nc.dram_tensor("gwlod", [128, NK * 8], BF16, kind="ExternalInput")
    gb128 = nc.dram_tensor("gb128", [128, 8], F32, kind="ExternalInput")
    web = nc.dram_tensor("web", [NWT, 128, WPACK * 1000], BF16,
                         kind="ExternalInput")
    be128 = nc.dram_tensor("be128", [128, C], F32, kind="ExternalInput")
    sel = nc.dram_tensor("sel", [128, 8], F32, kind="ExternalInput")
    out16 = nc.dram_tensor("out16", [SH, C], F32, kind="ExternalOutput")

    with TileContext(nc) as tc:
        with (
            tc.tile_pool(name="consts", bufs=1) as cp,
            tc.tile_pool(name="wbf", bufs=WBUFS) as wbf,
            tc.tile_pool(name="dram", bufs=1, space="DRAM") as dp,
        ):
            # ---- constants into SBUF (scalar: conv-critical; sync: rest) ----
            w1sb = cp.tile([36, 128], F32R, tag="w1sb")
            nc.scalar.dma_start(w1sb[:], w1bd[:, :])
            b1sb = cp.tile([128, 1], F32, tag="b1sb")
            nc.scalar.dma_start(b1sb[:], b1[:, :])
            b2sb = cp.tile([128, 1], F32, tag="b2sb")
            nc.scalar.dma_start(b2sb[:], b2[:, :])
            w2sb = cp.tile([128, 1152], F32R, tag="w2sb")
            nc.sync.dma_start(w2sb[:], w2bd[:, :])
            idsb = cp.tile([128, 128], F32, tag="idsb")
            make_identity(nc, idsb[:])
            # gate weights host-rearranged: col k*8+j = gw[128k+p, j],
            # split hi/lo bf16 (gwhi + gwlo == gw to ~2^-16)
            gwhi = cp.tile([128, NK * 8], BF16, tag="gwhi")
            nc.sync.dma_start(gwhi[:], gwhid[:, :])
            gwlo = cp.tile([128, NK * 8], BF16, tag="gwlo")
            nc.sync.dma_start(gwlo[:], gwlod[:, :])
            gbsb = cp.tile([128, 8], F32, tag="gbsb")
            nc.sync.dma_start(gbsb[:], gb128[:, :])
            besb = cp.tile([128, C], F32, tag="besb")
            nc.sync.dma_start(besb[:], be128[:, :])
            selsb = cp.tile([128, 8], F32, tag="selsb")
            nc.sync.dma_start(selsb[:], sel[:, :])

            # ---- DRAM bounce buffers for collectives ----
            h_localA = dp.tile([8, D], BF16, tag="h_localA")
            h_localB = dp.tile([4, D], BF16, tag="h_localB")
            h_localC = dp.tile([4, D], BF16, tag="h_localC")
            cc_in = dp.tile([B, C], F32, tag="cc_in")
            cc_out = dp.tile([SH, C], F32, tag="cc_out")

            for _rep in range(repeat):
                h_allA = dp.tile([64, D], BF16, tag=f"h_allA{_rep}",
                                 addr_space="Shared")
                h_allB = dp.tile([32, D], BF16, tag=f"h_allB{_rep}",
                                 addr_space="Shared")
                h_allC = dp.tile([32, D], BF16, tag=f"h_allC{_rep}",
                                 addr_space="Shared")
                # ---- expert weight stream: packed bf16 tiles ----
                wbf_tiles = []
                _pumped = [0]

                def pump(n=3, _rep=_rep):
                    if not do_w:
                        return
                    hi = min(NWT, _pumped[0] + n)
                    for t in range(_pumped[0], hi):
                        wb = wbf.tile([128, WPACK * 1000], BF16, tag="wb")
                        nc.sync.dma_start(wb[:], web[t, :, :])
                        wbf_tiles.append(wb)
                    _pumped[0] = hi

                def ag_piece(idx, _rep=_rep):
                    # gather completed image blocks while conv continues
                    if do_ag:
                        loc = (h_localA, h_localB, h_localC)[idx]
                        out = (h_allA, h_allB, h_allC)[idx]
                        nc.gpsimd.collective_compute(
                            "AllGather", ALU.bypass, replica_groups=RG,
                            ins=[loc.opt()], outs=[out.opt()],
                        )

                # =========== conv trunk ===========
                if do_conv:
                    _conv_trunk(nc, tc, x16, w1sb, b1sb, w2sb, b2sb, idsb,
                                (h_localA, h_localB, h_localC),
                                pump=lambda: pump(3), mid=ag_piece)
                else:
                    ag_piece(0)
                    ag_piece(1)
                pump(NWT)

                # =========== AllGather h (bf16), last image block ===========
                ag_piece(2)

                if variant not in ("conv_only", "conv_ag"):
                    _phase45(nc, tc, do_w, do_rs, wbf_tiles, idsb, gwhi,
                             gwlo, gbsb, besb, selsb,
                             (h_allA, h_allB, h_allC),
                             cc_in, cc_out, out16)

    nc.compile()
    return nc


_NC_CACHE = None


def _get_program():
    global _NC_CACHE
    if _NC_CACHE is None:
        _NC_CACHE = build_program()
    return _NC_CACHE


def _pack_web(expert_w_e):
    """[D, C] fp32 -> [NWT, 128, WPACK*1000] bf16 (chunks packed along rows)."""
    import ml_dtypes
    ew = np.asarray(expert_w_e, np.float32).astype(ml_dtypes.bfloat16)
    out = np.zeros((NWT, 128, WPACK * 1000), ml_dtypes.bfloat16)
    for t in range(NWT):
        for i in range(WPACK):
            k = t * WPACK + i
            if k >= NK:
                break
            out[t, :, i * 1000:(i + 1) * 1000] = ew[k * 128:(k + 1) * 128, :]
    return out


def make_in_maps(x, conv1_w, conv1_b, conv2_w, conv2_b,
                 gate_w, gate_b, expert_w, expert_b):
    x = np.asarray(x, np.float32).reshape(B, 4096)
    w1 = np.asarray(conv1_w, np.float32).reshape(9, 32)
    w1bd = np.zeros((36, 128), np.float32)
    for j in range(4):
        w1bd[9 * j:9 * j + 9, 32 * j:32 * j + 32] = w1
    b1 = np.ascontiguousarray(
        np.tile(np.asarray(conv1_b, np.float32), 4).reshape(128, 1))
    # conv2 block-diag-2: per tap [64, 128]: rows 0:32 img-even in-ch
    # (cols 0:64 out), rows 32:64 img-odd; duplicated to rows 64:128.
    w2 = np.asarray(conv2_w, np.float32).reshape(9, 32, 64)
    w2bd = np.zeros((128, 1152), np.float32)
    for tap in range(9):
        blk = np.zeros((64, 128), np.float32)
        blk[0:32, 0:64] = w2[tap]
        blk[32:64, 64:128] = w2[tap]
        w2bd[0:64, 128 * tap:128 * tap + 128] = blk
        w2bd[64:128, 128 * tap:128 * tap + 128] = blk
    b2 = np.ascontiguousarray(
        np.tile(np.asarray(conv2_b, np.float32), 2).reshape(128, 1))
    # gate weights pre-arranged [128, 98*8]: col k*8+j = gw[128k+p, j],
    # split into hi/lo bf16 halves (exact to ~2^-16 relative)
    import ml_dtypes
    gwf = np.asarray(gate_w, np.float32)
    gwre = np.ascontiguousarray(
        gwf.reshape(NK, 128, 8).transpose(1, 0, 2).reshape(128, NK * 8))
    gwhi = gwre.astype(ml_dtypes.bfloat16)
    gwlo = (gwre - gwhi.astype(np.float32)).astype(ml_dtypes.bfloat16)
    gb128 = np.ascontiguousarray(
        np.broadcast_to(np.asarray(gate_b, np.float32), (128, 8)))
    ew = np.asarray(expert_w, np.float32)
    eb = np.asarray(expert_b, np.float32)
    in_maps = []
    for r in range(E):
        onehot = np.zeros((1, 8), np.float32)
        onehot[0, r] = 1.0
        in_maps.append({
            "x16": np.ascontiguousarray(x[r * SH:(r + 1) * SH]),
            "w1bd": w1bd, "b1": b1, "w2bd": w2bd, "b2": b2,
            "gwhid": gwhi, "gwlod": gwlo, "gb128": gb128,
            "web": _pack_web(ew[r]),
            "be128": np.ascontiguousarray(
                np.broadcast_to(eb[r], (128, C))),
            "sel": np.ascontiguousarray(np.broadcast_to(onehot, (128, 8))),
        })
    return in_maps


def kernel(**inputs):
    nc = _get_program()
    in_maps = make_in_maps(**inputs)
    res = run_bass_kernel_spmd(nc, in_maps, core_ids=list(range(E)))
    return np.concatenate([res.results[r]["out16"] for r in range(E)], axis=0)
